# revision 16
# baseline (speedup 1.0000x reference)
"""Trainium2 Bass kernel for nn_Attention_33354716021131 (v2).

Dense GQA attention (B=2, S=2048, D=4096, 32 q-heads / 8 kv-heads, head_dim
128, RoPE, causal softmax) tensor-parallel across 8 NeuronCores.

Per core c: q-heads 4c..4c+3 (kv-head c) -> wq/wk/wv column shards, wo column
shard; host passes x pre-transposed (xT [D, T], bf16) to every core, so there
is no on-device input transpose and no input collective.  The only collectives
are two AllGathers (one per batch) of the attention outputs oT (bf16).

Pipeline per core:
  QKV   x-stationary matmuls produce q/k/v in natural [token, feat] layout
        (256-token granules, PSUM: 2x q-bank + 2x kv-bank), RoPE applied on
        the free axis with plain DVE ops, then q/k are PE-transposed into
        qT/kT [d, token]; v stays natural.  bf16 inputs, fp32 PSUM.
  ATTN  per (head, 512-query block): sT = kT_tile^T qT (fp32r), pT =
        exp(sT*scale) on ScalarE, causal tri-mask on diagonal tiles, oT +=
        v_nat_tile^T pT, denominators via ones-matmul; normalize with DVE
        reciprocal + partition-broadcast multiply (GpSimd only runs the two
        collectives).  Output oT written bf16.
  AG    AllGather oT [512, 2048] -> oT_F [4096, 2048] per batch (bf16).
  WO    strip-stationary: load oT_F row-strips [128 f, 512 t] (contiguous 1KB
        lines), psy[tti] += strip_chunk^T wo_chunk accumulated over 32 feature
        chunks; 4 token-tiles per group, PSUM double-buffered (8 banks).
All matmuls run at 1 cycle/row (bf16 or fp32r with free >= 256).
"""
import math
import os

import numpy as np

N_CORES = 8
B = 2
S = 2048
DM = 4096
N_HEADS = 32
HD = 128
NQH = N_HEADS // N_CORES          # 4 q heads per core
HDQ = NQH * HD                    # 512
T = B * S                         # 4096 tokens
KC = DM // 128                    # 32 contraction chunks
NG = S // 256                     # 8 granules (256 tokens) per batch
NGT = S // 128                    # 16 token tiles per batch
QB = 512                          # query block for attention
NQB = S // QB                     # 4
SCALE = 1.0 / math.sqrt(HD)
ROPE_THETA = 10000.0

_CACHE = {}


def _consts():
    j = np.arange(HD // 2)
    inv = 1.0 / (ROPE_THETA ** (2 * j / HD))          # [64]
    pos = np.arange(S).reshape(NGT, 128)              # [16, 128]
    ang = pos[:, :, None] * inv[None, None, :]        # [16, 128, 64]
    cos = np.cos(ang).astype(np.float32)
    sin = np.sin(ang).astype(np.float32)
    # [128 part, 16 tiles, 4 head-reps, 64 freqs] -> [128, 4096]
    cos4 = np.tile(cos.transpose(1, 0, 2)[:, :, None, :], (1, 1, NQH, 1))
    sin4 = np.tile(sin.transpose(1, 0, 2)[:, :, None, :], (1, 1, NQH, 1))
    cos4 = np.ascontiguousarray(cos4.reshape(128, NGT * NQH * 64))
    sin4 = np.ascontiguousarray(sin4.reshape(128, NGT * NQH * 64))
    tri = (np.arange(128)[:, None] <= np.arange(128)[None, :]).astype(np.float32)
    ident = np.eye(128, dtype=np.float32)
    ones = np.ones((128, 128), np.float32)
    return cos4, sin4, tri, ident, ones


def _build(sim=False):
    import concourse.mybir as mybir
    import concourse.tile as tile
    from concourse import bacc

    F32 = mybir.dt.float32
    F32R = mybir.dt.float32r
    BF16 = mybir.dt.bfloat16

    nc = bacc.Bacc("TRN2", target_bir_lowering=False, debug=False,
                   num_devices=N_CORES)

    xt = nc.dram_tensor("xt", [DM, T], BF16, kind="ExternalInput")
    wq = nc.dram_tensor("wq", [DM, HDQ], BF16, kind="ExternalInput")
    wkv = nc.dram_tensor("wkv", [DM, 256], BF16, kind="ExternalInput")
    wo = nc.dram_tensor("wo", [DM, HDQ], BF16, kind="ExternalInput")
    cosc = nc.dram_tensor("cosc", [128, NGT * 256], BF16, kind="ExternalInput")
    sinc = nc.dram_tensor("sinc", [128, NGT * 256], BF16, kind="ExternalInput")
    tric = nc.dram_tensor("tric", [128, 128], BF16, kind="ExternalInput")
    identc = nc.dram_tensor("identc", [128, 128], BF16, kind="ExternalInput")
    onesc = nc.dram_tensor("onesc", [128, 128], BF16, kind="ExternalInput")

    y = nc.dram_tensor("y", [T, HDQ], F32, kind="ExternalOutput")

    rg = [list(range(N_CORES))]

    with tile.TileContext(nc) as tc:
        with (
            tc.tile_pool(name="dram", bufs=1, space="DRAM") as dram,
            tc.tile_pool(name="const", bufs=1) as cp,
        ):
            cos_sb = cp.tile([128, NGT * 256], BF16, tag="cos")
            sin_sb = cp.tile([128, NGT * 256], BF16, tag="sin")
            tri_sb = cp.tile([128, 128], BF16, tag="tri")
            id_sb = cp.tile([128, 128], BF16, tag="id")
            ones_sb = cp.tile([128, 128], BF16, tag="ones")
            nc.scalar.dma_start(out=cos_sb[:], in_=cosc.ap())
            nc.scalar.dma_start(out=sin_sb[:], in_=sinc.ap())
            nc.scalar.dma_start(out=tri_sb[:], in_=tric.ap())
            nc.scalar.dma_start(out=id_sb[:], in_=identc.ap())
            nc.scalar.dma_start(out=ones_sb[:], in_=onesc.ap())

            oT_h = [[dram.tile([HDQ, QB], BF16, name=f"oT_h{b}_{qb}")
                     for qb in range(NQB)] for b in range(B)]
            oT_F = [[dram.tile([DM, QB], BF16,
                               addr_space="Local" if sim else "Shared",
                               name=f"oT_F{b}_{qb}") for qb in range(NQB)]
                    for b in range(B)]

            with tc.tile_pool(name="wqkv", bufs=1) as wpool:
                wq_sbs, wkv_sbs = [], []
                for g4 in range(4):
                    ksl = slice(g4 * 8, (g4 + 1) * 8)
                    wq_t = wpool.tile([128, 8 * HDQ], BF16, tag=f"wq{g4}",
                                      name=f"wq{g4}")
                    wkv_t = wpool.tile([128, 8 * 256], BF16, tag=f"wkv{g4}",
                                       name=f"wkv{g4}")
                    nc.sync.dma_start(
                        out=wq_t[:].rearrange("p (kc d) -> p kc d", kc=8),
                        in_=wq.ap().rearrange("(kc p) d -> p kc d",
                                              p=128)[:, ksl, :],
                    )
                    nc.sync.dma_start(
                        out=wkv_t[:].rearrange("p (kc d) -> p kc d", kc=8),
                        in_=wkv.ap().rearrange("(kc p) d -> p kc d",
                                               p=128)[:, ksl, :],
                    )
                    wq_sbs.append(wq_t)
                    wkv_sbs.append(wkv_t)
                wo_sb = wpool.tile([128, KC * HDQ], BF16, tag="wo")
                nc.scalar.dma_start(
                    out=wo_sb[:].rearrange("p (kc d) -> p kc d", kc=KC),
                    in_=wo.ap().rearrange("(kc p) d -> p kc d", p=128),
                )

                with (
                    tc.tile_pool(name="batch", bufs=1) as bp,
                    tc.tile_pool(name="xtp", bufs=28) as xtp,
                    tc.tile_pool(name="rwp", bufs=2) as rwp,
                    tc.tile_pool(name="qrp", bufs=2) as qrp,
                    tc.tile_pool(name="tmp", bufs=2) as tmp,
                    tc.tile_pool(name="wa", bufs=2) as wa,
                    tc.tile_pool(name="ptp", bufs=3) as ptp,
                    tc.tile_pool(name="accp", bufs=2) as accp,
                    tc.tile_pool(name="stp", bufs=14) as stp,
                    tc.tile_pool(name="ywp", bufs=2) as ywp,
                ):
                    qTall = bp.tile([128, NQH * S], BF16, tag="qTall")
                    kT = bp.tile([128, S], BF16, tag="kT")
                    v_nat = bp.tile([128, S], BF16, tag="v_nat")
                    pools = dict(xtp=xtp, rwp=rwp, qrp=qrp, tmp=tmp,
                                 wa=wa, ptp=ptp, accp=accp)

                    for b in range(B):
                        _emit_qkv(nc, tc, b, dict(
                            mybir=mybir, F32=F32, F32R=F32R, BF16=BF16,
                            xt=xt, wq_sbs=wq_sbs, wkv_sbs=wkv_sbs,
                            cos_sb=cos_sb, sin_sb=sin_sb, id_sb=id_sb,
                            qTall=qTall, kT=kT, v_nat=v_nat,
                            pools=pools,
                        ))
                        _emit_attn(nc, tc, b, dict(
                            mybir=mybir, F32=F32, F32R=F32R, BF16=BF16,
                            qTall=qTall, kT=kT, v_nat=v_nat,
                            tri_sb=tri_sb, ones_sb=ones_sb, oT_h=oT_h,
                            oT_F=oT_F, rg=rg, sim=sim,
                            pools=pools,
                        ))

                    # ---------- WO projection ----------
                    with tc.tile_pool(name="ps_y", bufs=2,
                                      space="PSUM") as ps_y:
                      for b in range(B):
                        for tg in range(S // 512):
                            psy = [ps_y.tile([128, HDQ], F32, tag=f"psy{i}",
                                             name=f"psy{i}")
                                   for i in range(4)]
                            for hc in range(KC):
                                strip = stp.tile([128, 512], BF16,
                                                 tag="strip")
                                seng = nc.sync if hc % 2 == 0 else nc.scalar
                                seng.dma_start(
                                    out=strip[:],
                                    in_=oT_F[b][tg][:]
                                    [hc * 128:(hc + 1) * 128, :],
                                )
                                for tti in range(4):
                                    nc.tensor.matmul(
                                        psy[tti][:],
                                        strip[:, tti * 128:(tti + 1) * 128],
                                        wo_sb[:, hc * HDQ:(hc + 1) * HDQ],
                                        start=(hc == 0), stop=(hc == KC - 1),
                                    )
                            for tti in range(4):
                                y_sb = ywp.tile([128, HDQ], F32, tag="y_sb")
                                nc.scalar.copy(y_sb[:], psy[tti][:])
                                row = b * S + tg * 512 + tti * 128
                                nc.scalar.dma_start(
                                    out=y.ap()[row:row + 128, :],
                                    in_=y_sb[:])

    nc.compile()
    return nc


def _emit_qkv(nc, tc, b, t):
    F32, F32R, BF16 = t["F32"], t["F32R"], t["BF16"]
    xt, wq_sbs, wkv_sbs = t["xt"], t["wq_sbs"], t["wkv_sbs"]
    cos_sb, sin_sb, id_sb = t["cos_sb"], t["sin_sb"], t["id_sb"]
    qTall, kT, v_nat = t["qTall"], t["kT"], t["v_nat"]

    xtp, rwp, qrp, tmp = (t["pools"][k] for k in ("xtp", "rwp", "qrp", "tmp"))
    with (
        tc.tile_pool(name=f"ps_acc{b}", bufs=1, space="PSUM") as ps_acc,
        tc.tile_pool(name=f"ps_T{b}", bufs=1, space="PSUM") as ps_T,
    ):
        def emit_T(prev):
            if prev is None:
                return
            g0, q_rots, k_rots = prev
            pos = g0 * 256
            for tti in range(2):
                tq = ps_T.tile([128, HDQ], BF16, tag=f"tq{tti}",
                               name=f"tq{tti}", padded_shape=[128, 1024])
                for h in range(NQH):
                    nc.tensor.transpose(
                        tq[:, h * 128:(h + 1) * 128],
                        q_rots[tti][:, h * 128:(h + 1) * 128],
                        id_sb[:],
                    )
                nc.scalar.copy(
                    qTall[:].rearrange("p (h s) -> p h s", h=NQH)
                    [:, :, pos + tti * 128:pos + (tti + 1) * 128],
                    tq[:].rearrange("p (h t) -> p h t", h=NQH),
                )
            tk = ps_T.tile([128, 256], BF16, tag="tk",
                           padded_shape=[128, 1024])
            for tti in range(2):
                nc.tensor.transpose(
                    tk[:, tti * 128:(tti + 1) * 128],
                    k_rots[tti][:],
                    id_sb[:],
                )
            nc.scalar.copy(kT[:, pos:pos + 256], tk[:])

        prev = None
        for g in range(NG):
            tok0 = b * S + g * 256
            psq = [ps_acc.tile([128, HDQ], F32, tag=f"psq{i}", name=f"psq{i}")
                   for i in range(2)]
            pskv = [ps_acc.tile([128, 256], F32, tag=f"pskv{i}",
                                name=f"pskv{i}", padded_shape=[128, 512])
                    for i in range(2)]
            for kcp in range(KC // 2):
                xt2 = xtp.tile([128, 512], BF16, tag="xt2")
                eng = nc.sync if kcp % 2 == 0 else nc.scalar
                eng.dma_start(
                    out=xt2[:].rearrange("p (c t) -> p c t", c=2),
                    in_=xt.ap()[kcp * 256:(kcp + 1) * 256, tok0:tok0 + 256]
                    .rearrange("(c p) t -> p c t", p=128),
                )
                for c2 in range(2):
                    kc = kcp * 2 + c2
                    for tti in range(2):
                        lhsT = xt2[:, c2 * 256 + tti * 128:
                                   c2 * 256 + (tti + 1) * 128]
                        nc.tensor.matmul(
                            psq[tti][:], lhsT,
                            wq_sbs[kc // 8][:, (kc % 8) * HDQ:
                                            (kc % 8 + 1) * HDQ],
                            start=(kc == 0), stop=(kc == KC - 1),
                        )
                        nc.tensor.matmul(
                            pskv[tti][:], lhsT,
                            wkv_sbs[kc // 8][:, (kc % 8) * 256:
                                             (kc % 8 + 1) * 256],
                            start=(kc == 0), stop=(kc == KC - 1),
                        )

            emit_T(prev)

            q_rots, k_rots = [], []
            for tti in range(2):
                gt = g * 2 + tti
                rq = rwp.tile([128, HDQ], BF16, tag=f"rq{tti}", name=f"rq{tti}")
                nc.scalar.copy(rq[:], psq[tti][:])
                rkv = rwp.tile([128, 256], BF16, tag=f"rkv{tti}",
                               name=f"rkv{tti}")
                nc.scalar.copy(rkv[:], pskv[tti][:])
                nc.scalar.copy(v_nat[:, gt * 128:(gt + 1) * 128],
                               rkv[:, 128:256])

                csl = slice(gt * 256, gt * 256 + 256)
                ksl = slice(gt * 256, gt * 256 + 64)
                q_rot = qrp.tile([128, HDQ], BF16, tag=f"qr{tti}",
                                 name=f"qr{tti}")
                x0 = rq[:].rearrange("p (d two) -> p d two", two=2)[:, :, 0]
                x1 = rq[:].rearrange("p (d two) -> p d two", two=2)[:, :, 1]
                r0 = q_rot[:].rearrange("p (d two) -> p d two", two=2)[:, :, 0]
                r1 = q_rot[:].rearrange("p (d two) -> p d two", two=2)[:, :, 1]
                m0 = tmp.tile([128, 256], BF16, tag="m0", name="m0")
                m1 = tmp.tile([128, 256], BF16, tag="m1", name="m1")
                nc.vector.tensor_mul(m0[:], x0, cos_sb[:, csl])
                nc.vector.tensor_mul(m1[:], x1, sin_sb[:, csl])
                nc.vector.tensor_sub(r0, m0[:], m1[:])
                m2 = tmp.tile([128, 256], BF16, tag="m0", name="m2")
                m3 = tmp.tile([128, 256], BF16, tag="m1", name="m3")
                nc.vector.tensor_mul(m2[:], x0, sin_sb[:, csl])
                nc.vector.tensor_mul(m3[:], x1, cos_sb[:, csl])
                nc.vector.tensor_add(r1, m2[:], m3[:])

                k_rot = qrp.tile([128, 128], BF16, tag=f"kr{tti}",
                                 name=f"kr{tti}")
                kx0 = rkv[:, 0:128].rearrange("p (d two) -> p d two",
                                              two=2)[:, :, 0]
                kx1 = rkv[:, 0:128].rearrange("p (d two) -> p d two",
                                              two=2)[:, :, 1]
                kr0 = k_rot[:].rearrange("p (d two) -> p d two",
                                         two=2)[:, :, 0]
                kr1 = k_rot[:].rearrange("p (d two) -> p d two",
                                         two=2)[:, :, 1]
                km0 = tmp.tile([128, 64], BF16, tag="km0", name="km0")
                km1 = tmp.tile([128, 64], BF16, tag="km1", name="km1")
                nc.vector.tensor_mul(km0[:], kx0, cos_sb[:, ksl])
                nc.vector.tensor_mul(km1[:], kx1, sin_sb[:, ksl])
                nc.vector.tensor_sub(kr0, km0[:], km1[:])
                km2 = tmp.tile([128, 64], BF16, tag="km0", name="km2")
                km3 = tmp.tile([128, 64], BF16, tag="km1", name="km3")
                nc.vector.tensor_mul(km2[:], kx0, sin_sb[:, ksl])
                nc.vector.tensor_mul(km3[:], kx1, cos_sb[:, ksl])
                nc.vector.tensor_add(kr1, km2[:], km3[:])
                q_rots.append(q_rot)
                k_rots.append(k_rot)

            prev = (g, q_rots, k_rots)
        emit_T(prev)


def _emit_attn(nc, tc, b, t):
    mybir = t["mybir"]
    F32, F32R, BF16 = t["F32"], t["F32R"], t["BF16"]
    qTall, kT, v_nat = t["qTall"], t["kT"], t["v_nat"]
    tri_sb, ones_sb, oT_h = t["tri_sb"], t["ones_sb"], t["oT_h"]

    wp, ptp = t["pools"]["wa"], t["pools"]["ptp"]
    accp = t["pools"]["accp"]
    oT_F, rg, sim = t["oT_F"], t["rg"], t["sim"]
    with (
        tc.tile_pool(name=f"ps_s{b}", bufs=3, space="PSUM") as ps_s,
        tc.tile_pool(name=f"ps_o{b}", bufs=3, space="PSUM") as ps_o,
        tc.tile_pool(name=f"ps_sum{b}", bufs=2, space="PSUM") as ps_sum,
    ):
        for qb in range(NQB):
            for h in range(NQH):
                q0 = qb * QB
                kt_max = (q0 + QB) // 128 - 1
                oT = ps_o.tile([128, QB], F32, tag="oT")
                sums = ps_sum.tile([128, QB], F32, tag="sums")
                acc = accp.tile([128, QB], BF16, tag="acc", name="acc")

                sTs = {}

                def emit_s(kt):
                    off = max(0, kt * 128 - q0)
                    qs = slice(h * S + q0 + off, h * S + q0 + QB)
                    sT = ps_s.tile([128, QB], F32, tag="sT", name="sT")
                    nc.tensor.matmul(
                        sT[:, off:QB],
                        kT[:, kt * 128:(kt + 1) * 128],
                        qTall[:, qs],
                        start=True, stop=True,
                    )
                    sTs[kt] = (sT, off)

                emit_s(0)
                if kt_max >= 1:
                    emit_s(1)
                for kt in range(kt_max + 1):
                    if kt + 2 <= kt_max:
                        emit_s(kt + 2)
                    sT, off = sTs.pop(kt)
                    psl = slice(off, QB)
                    pT = ptp.tile([128, QB], BF16, tag="pT", name="pT")
                    nc.scalar.activation(
                        pT[:, psl], sT[:, psl],
                        mybir.ActivationFunctionType.Exp,
                        scale=SCALE,
                    )
                    if kt * 128 >= q0:
                        nc.vector.tensor_mul(
                            pT[:, off:off + 128],
                            pT[:, off:off + 128],
                            tri_sb[:],
                        )
                    nc.tensor.matmul(
                        oT[:, psl],
                        v_nat[:, kt * 128:(kt + 1) * 128],
                        pT[:, psl],
                        start=(kt == 0), stop=(kt == kt_max),
                    )
                    if kt == 0:
                        nc.vector.tensor_copy(acc[:], pT[:])
                    else:
                        nc.vector.tensor_add(acc[:, psl], acc[:, psl],
                                             pT[:, psl])
                nc.tensor.matmul(
                    sums[:], ones_sb[:], acc[:],
                    start=True, stop=True,
                )
                rec = wp.tile([128, QB], F32, tag="rec")
                scr = wp.tile([128, QB], F32, tag="scr")
                nc.vector.reciprocal_approx_accurate(rec[:], sums[:],
                                                     scr[:])
                oT_sb = wp.tile([128, QB], BF16, tag="oT_sb")
                nc.vector.tensor_mul(oT_sb[:], oT[:], rec[:])
                nc.scalar.dma_start(
                    out=oT_h[b][qb][:][h * 128:(h + 1) * 128, :],
                    in_=oT_sb[:],
                )
            if not sim:
                nc.gpsimd.collective_compute(
                    "AllGather", mybir.AluOpType.bypass,
                    replica_groups=rg,
                    ins=[oT_h[b][qb][:].opt()],
                    outs=[oT_F[b][qb][:].opt()],
                )
            else:
                for c in range(N_CORES):
                    nc.sync.dma_start(
                        out=oT_F[b][qb][:][c * HDQ:(c + 1) * HDQ, :],
                        in_=oT_h[b][qb][:],
                    )


def _in_maps(x, wq, wk, wv, wo):
    import concourse.mybir as mybir
    np_bf16 = mybir.dt.np(mybir.dt.bfloat16)

    x2 = np.asarray(x, dtype=np.float32).reshape(T, DM)
    xT = np.ascontiguousarray(x2.T).astype(np_bf16)
    cos4, sin4, tri, ident, ones = _consts()
    wq = np.asarray(wq, np.float32)
    wk = np.asarray(wk, np.float32)
    wv = np.asarray(wv, np.float32)
    wo = np.asarray(wo, np.float32)
    maps = []
    for c in range(N_CORES):
        qsl = slice(c * HDQ, (c + 1) * HDQ)
        ksl = slice(c * HD, (c + 1) * HD)
        wkv_c = np.concatenate([wk[:, ksl], wv[:, ksl]], axis=1)
        maps.append({
            "xt": xT,
            "wq": np.ascontiguousarray(wq[:, qsl]).astype(np_bf16),
            "wkv": np.ascontiguousarray(wkv_c).astype(np_bf16),
            "wo": np.ascontiguousarray(wo[:, qsl]).astype(np_bf16),
            "cosc": cos4.astype(np_bf16), "sinc": sin4.astype(np_bf16),
            "tric": tri.astype(np_bf16),
            "identc": ident.astype(np_bf16), "onesc": ones.astype(np_bf16),
        })
    return maps


def kernel(x, wq, wk, wv, wo, start_pos=0, **_unused):
    from concourse import bass_utils

    assert int(np.asarray(start_pos)) == 0
    in_maps = _in_maps(x, wq, wk, wv, wo)

    if "nc" not in _CACHE:
        _CACHE["nc"] = _build()
    nc = _CACHE["nc"]

    res = bass_utils.run_bass_kernel_spmd(
        nc, in_maps, core_ids=list(range(N_CORES)),
        trace=bool(int(os.environ.get("KERNEL_TRACE", "0") or 0)),
    )
    _CACHE["last_result"] = res

    out = np.empty((T, DM), np.float32)
    for c in range(N_CORES):
        out[:, c * HDQ:(c + 1) * HDQ] = res.results[c]["y"]
    return out.reshape(B, S, DM)
